# revision 1
# baseline (speedup 1.0000x reference)
"""Neural CDE (RK4 scan over spline-interpolated path) on 8 Trainium2 cores.

Strategy:
  - Pure batch data-parallelism: B=8192 -> 8 cores x 1024 elements.
  - Host precomputes the per-step, per-RK4-stage spline derivatives
    dX (b + c*f + d*f^2 at f in {0, dt/2, dt}), pre-scaled by the RK4
    step factors so the device glue is pure adds:
        k1' = (dt/2) k1   (z2 = z + k1')
        k2' = (dt/2) k2   (z3 = z + k2')
        k3' = dt     k3   (z4 = z + k3')
        k4' = (dt/6) k4
        z_next = z + (k1' + k3')/3 + (2/3) k2' + k4'
  - On device, per core: 2 gangs x 4 tiles x 128 batch.  Per stage:
      PE transpose z (batch-major -> feature-major), mm1 (W1 stationary,
      bias row folded via ones-row augmentation, K=33), tanh on ACT,
      mm2 (h as stationary, W2 moving) -> g (batch-major, 192 cols),
      einsum g . dx via DVE tensor_tensor with a stride-0 broadcast view
      + grouped tensor_reduce over the 6 input channels,
      RK4 glue adds on GPSIMD.
  - Final readout (z @ Wl + bl) on host (tiny).
"""

import numpy as np

B, L, C, H, MID = 8192, 256, 6, 32, 128
NCORES = 8
PCORE = B // NCORES        # 1024
P = 128
NTILES = PCORE // P        # 8
NGANGS = 2
TPG = NTILES // NGANGS     # 4
HC = H * C                 # 192
NSTEPS = L - 1             # 255

_PROG_CACHE = {}


def _dedup_sync_waits(nc):
    """Merge duplicate semaphore waits on each instruction.

    Tile's sem-assignment can emit two waits on the same semaphore (loop-entry
    + in-body); walrus codegen has a small per-instruction wait capacity, so
    collapse duplicates (sem >= a AND sem >= b  ==  sem >= max(a, b))."""
    import concourse.mybir as mybir

    for f in nc.m.functions:
        for bb in f.blocks:
            for inst in bb.instructions:
                si = inst.sync_info
                if si is None or not si.on_wait or len(si.on_wait) <= 1:
                    continue
                seen = {}
                order = []
                mergeable = True
                for w in si.on_wait:
                    key = (w.sync_type, w.id, w.wait_mode, w.wait_reg)
                    if key in seen and w.wait_mode == "sem-ge-imm":
                        if (w.wait_value or 0) > (seen[key].wait_value or 0):
                            seen[key] = w
                    elif key in seen:
                        mergeable = False
                        break
                    else:
                        seen[key] = w
                        order.append(key)
                if mergeable and len(order) != len(si.on_wait):
                    inst.sync_info = mybir.SyncInfo(
                        on_wait=[seen[k] for k in order], on_update=si.on_update
                    )


def _spill_sync_waits(nc):
    """Walrus codegen has a small per-instruction sync-wait capacity
    (~3 for LDW-carrying Matmult, ~4 elsewhere).  Move excess waits onto
    same-engine NoOps inserted immediately before the instruction —
    engine program order makes this equivalent."""
    import concourse.mybir as mybir

    def limit_for(inst):
        return 1

    for f in nc.m.functions:
        for bb in f.blocks:
            il = bb.instructions
            new = []
            changed = False
            for inst in il:
                si = inst.sync_info
                waits = list(si.on_wait) if (si and si.on_wait) else []
                lim = limit_for(inst)
                if len(waits) > lim:
                    excess = waits[: len(waits) - lim]
                    keep = waits[len(waits) - lim :]
                    for i in range(0, len(excess), 1):
                        nop = mybir.InstNoOp(
                            name=f"{inst.name}-wsp{i}",
                            engine=inst.engine,
                            sync_info=mybir.SyncInfo(
                                on_wait=excess[i : i + 1], on_update=[]
                            ),
                            bass_nofuse=True,
                        )
                        new.append(nop)
                    inst.sync_info = mybir.SyncInfo(
                        on_wait=keep, on_update=si.on_update
                    )
                    changed = True
                new.append(inst)
            if changed:
                bb.instructions = new


def _build_program(nsteps, with_b2, use_for_i=True, spill=True):
    from contextlib import ExitStack

    import concourse.bass as bass
    import concourse.mybir as mybir
    from concourse.tile import TileContext

    f32 = mybir.dt.float32
    AF = mybir.ActivationFunctionType
    ALU = mybir.AluOpType

    nc = bass.Bass()
    CB = 2 * P + HC + (HC if with_b2 else 0)  # w1aug(128) + ident(128) + w2(192) [+ b2bc(192)]
    dx_d = nc.declare_dram_parameter("dx", [nsteps, P, NTILES * 4 * C], f32, False)
    cb_d = nc.declare_dram_parameter("cblob", [P, CB], f32, False)
    zout_d = nc.declare_dram_parameter("zout", [P, NTILES * H], f32, True)

    with TileContext(nc) as tc, ExitStack() as ctx:
        const = ctx.enter_context(tc.tile_pool(name="const", bufs=1))
        state = ctx.enter_context(tc.tile_pool(name="state", bufs=1))
        dxpool = ctx.enter_context(tc.tile_pool(name="dxp", bufs=2))
        hpool = ctx.enter_context(tc.tile_pool(name="hsb", bufs=2))
        tpool = ctx.enter_context(tc.tile_pool(name="tsb", bufs=4))
        kpool = ctx.enter_context(tc.tile_pool(name="ksb", bufs=2))
        zspool = ctx.enter_context(tc.tile_pool(name="zs", bufs=2))
        scpool = ctx.enter_context(tc.tile_pool(name="sc", bufs=2))
        hppool = ctx.enter_context(tc.tile_pool(name="hp", bufs=2, space="PSUM"))
        gppool = ctx.enter_context(tc.tile_pool(name="gp", bufs=4, space="PSUM"))
        tppool = ctx.enter_context(tc.tile_pool(name="tp", bufs=2, space="PSUM"))

        cb_s = const.tile([P, CB], f32, tag="cblob")
        nc.sync.dma_start(cb_s[:], cb_d[:])
        w1_s = cb_s[0 : H + 1, 0:MID]
        id_s = cb_s[:, P : 2 * P]
        w2_s = cb_s[:, 2 * P : 2 * P + HC]
        if with_b2:
            b2v = (
                cb_s[:, 2 * P + HC : 2 * P + 2 * HC]
                .rearrange("p (h i) -> p h i", i=C)
                .unsqueeze(1)
                .broadcast_to((P, 2, H, C))
            )

        # Warm reads: consume every const on PE once before the loop so its
        # observed tick for the cblob DMA advances and in-loop matmuls don't
        # burn a wait slot re-waiting on it every iteration.
        warm_ps = tppool.tile([P, P], f32, tag="tp")
        nc.tensor.matmul(warm_ps[:], id_s, w2_s[:, 0:P], start=True, stop=True)
        zz = []
        for g in range(NGANGS):
            zt_ = state.tile([P, TPG * H], f32, tag=f"zz{g}")
            nc.vector.memset(zt_[:], 0.0)
            zz.append(zt_)
        zTs = []
        for g in range(NGANGS):
            pair = []
            for nm in range(2):
                t = state.tile([H + 1, TPG * P], f32, tag=f"zT{g}_{nm}")
                nc.vector.memset(t[H : H + 1, :], 1.0)  # ones row (bias)
                pair.append(t)
            zTs.append(pair)

        def emit_step(dxt):
            for g in range(NGANGS):
                kk = kpool.tile([P, 4 * TPG * H], f32, tag=f"kk{g}")
                zstage = zz[g]
                for s in range(4):
                    # --- transpose stage-z to feature-major (33, 512) ---
                    tp_t = tppool.tile([H, TPG * P], f32, tag="tp")
                    for c in range(TPG):
                        nc.tensor.matmul(
                            tp_t[:, c * P : (c + 1) * P],
                            zstage[:, c * H : (c + 1) * H],
                            id_s,
                            start=(c == 0),
                            stop=(c == TPG - 1),
                            is_transpose=True,
                        )
                    zT = zTs[g][s % 2]
                    nc.scalar.activation(zT[0:H, :], tp_t[:], AF.Copy)
                    # --- mm1 + tanh -> hT (feature-major, SBUF) ---
                    hp_t = hppool.tile([MID, TPG * P], f32, tag="hp")
                    nc.tensor.matmul(hp_t[:], w1_s, zT[:], start=True, stop=True)
                    hT = hpool.tile([MID, TPG * P], f32, tag="hT")
                    nc.scalar.activation(hT[:], hp_t[:], AF.Tanh)
                    # --- mm2 (2 tiles per psum bank) + einsum ---
                    for j in range(2):
                        gp_t = gppool.tile([P, 2 * HC], f32, tag="gp")
                        for u in range(2):
                            c = 2 * j + u
                            nc.tensor.matmul(
                                gp_t[:, u * HC : (u + 1) * HC],
                                hT[:, c * P : (c + 1) * P],
                                w2_s,
                                start=(u == 0),
                                stop=(u == 1),
                            )
                        T0 = g * TPG + 2 * j
                        dxv = (
                            dxt[:]
                            .rearrange("p (t v i) -> p t v i", t=NTILES, v=4)[
                                :, T0 : T0 + 2, s, :
                            ]
                            .unsqueeze(2)
                            .broadcast_to((P, 2, H, C))
                        )
                        gv = gp_t[:].rearrange("p (u h i) -> p u h i", u=2, i=C)
                        t_t = tpool.tile([P, 2 * HC], f32, tag="tt")
                        tv = t_t[:].rearrange("p (u h i) -> p u h i", u=2, i=C)
                        if with_b2:
                            g2 = tpool.tile([P, 2 * HC], f32, tag="g2")
                            g2v = g2[:].rearrange("p (u h i) -> p u h i", u=2, i=C)
                            nc.vector.tensor_tensor(g2v, gv, b2v, ALU.add)
                            nc.vector.tensor_tensor(tv, g2v, dxv, ALU.mult)
                        else:
                            nc.vector.tensor_tensor(tv, gv, dxv, ALU.mult)
                        ksl = kk[
                            :, s * TPG * H + j * 2 * H : s * TPG * H + (j + 1) * 2 * H
                        ]
                        nc.vector.tensor_reduce(
                            ksl, tv, axis=mybir.AxisListType.X, op=ALU.add
                        )
                    if s < 3:
                        znew = zspool.tile([P, TPG * H], f32, tag=f"zst{g}")
                        nc.vector.tensor_tensor(
                            znew[:],
                            zz[g][:],
                            kk[:, s * TPG * H : (s + 1) * TPG * H],
                            ALU.add,
                        )
                        zstage = znew
                # --- RK4 combine: z += (k1+k3)/3 + (2/3) k2 + k4 ---
                k1 = kk[:, 0 * TPG * H : 1 * TPG * H]
                k2 = kk[:, 1 * TPG * H : 2 * TPG * H]
                k3 = kk[:, 2 * TPG * H : 3 * TPG * H]
                k4 = kk[:, 3 * TPG * H : 4 * TPG * H]
                a = scpool.tile([P, TPG * H], f32, tag=f"cmb{g}")
                nc.vector.tensor_tensor(a[:], k1, k3, ALU.add)
                bb = scpool.tile([P, TPG * H], f32, tag=f"cmb2{g}")
                nc.vector.scalar_tensor_tensor(
                    bb[:], a[:], 1.0 / 3.0, k4, ALU.mult, ALU.add
                )
                cc = scpool.tile([P, TPG * H], f32, tag=f"cmb3{g}")
                nc.vector.scalar_tensor_tensor(
                    cc[:], k2, 2.0 / 3.0, bb[:], ALU.mult, ALU.add
                )
                nc.vector.tensor_tensor(zz[g][:], cc[:], zz[g][:], ALU.add)

        if use_for_i:
            tc.strict_bb_all_engine_barrier()
            with tc.For_i(0, nsteps, 1, staggered_reset=True) as iv:
                dxt = dxpool.tile([P, NTILES * 4 * C], f32, tag="dx")
                nc.sync.dma_start(dxt[:], dx_d[bass.ds(iv, 1)].squeeze(0))
                emit_step(dxt)
        else:
            for it in range(nsteps):
                dxt = dxpool.tile([P, NTILES * 4 * C], f32, tag="dx")
                nc.gpsimd.dma_start(dxt[:], dx_d[it])
                emit_step(dxt)

        if use_for_i:
            tc.strict_bb_all_engine_barrier()
        for g in range(NGANGS):
            nc.sync.dma_start(zout_d[:, g * TPG * H : (g + 1) * TPG * H], zz[g][:])
    _dedup_sync_waits(nc)
    if spill:
        _spill_sync_waits(nc)
    return nc


def _get_program(nsteps, with_b2, use_for_i=True):
    key = (nsteps, with_b2, use_for_i)
    if key not in _PROG_CACHE:
        _PROG_CACHE[key] = _build_program(nsteps, with_b2, use_for_i)
    return _PROG_CACHE[key]


def _host_prep(times, coeff_b, coeff_c, coeff_d):
    """-> (nsteps, dxc) with dxc shaped (NCORES, nsteps, P, NTILES*4*C)."""
    times = np.asarray(times, np.float32)
    b_ = np.asarray(coeff_b, np.float32)
    c_ = np.asarray(coeff_c, np.float32)
    d_ = np.asarray(coeff_d, np.float32)
    dts = (times[1:] - times[:-1]).astype(np.float32)  # (nsteps,)
    nsteps = dts.shape[0]
    fm = (dts / 2).astype(np.float32)
    dx0 = b_
    dxm = b_ + c_ * fm[None, :, None] + d_ * (fm * fm)[None, :, None]
    dx1 = b_ + c_ * dts[None, :, None] + d_ * (dts * dts)[None, :, None]
    sA = (dts / 2)[None, :, None].astype(np.float32)
    sC = dts[None, :, None].astype(np.float32)
    sD = (dts / 6)[None, :, None].astype(np.float32)
    dxall = np.stack(
        [dx0 * sA, dxm * sA, dxm * sC, dx1 * sD], axis=2
    )  # (B, nsteps, 4, C)
    dxc = (
        dxall.reshape(NCORES, NTILES, P, nsteps, 4, C)
        .transpose(0, 3, 2, 1, 4, 5)
        .reshape(NCORES, nsteps, P, NTILES * 4 * C)
        .astype(np.float32)
    )
    return nsteps, np.ascontiguousarray(dxc)


def _make_cblob(W1, b1, W2, b2, with_b2):
    """(128, CB) const blob: [w1aug pad | I128 | W2 | b2bc?]."""
    CB = 2 * P + HC + (HC if with_b2 else 0)
    cb = np.zeros((P, CB), np.float32)
    cb[0:H, 0:MID] = W1
    cb[H, 0:MID] = b1
    cb[:, P : 2 * P] = np.eye(P, dtype=np.float32)
    cb[0:MID, 2 * P : 2 * P + HC] = W2
    if with_b2:
        cb[:, 2 * P + HC : 2 * P + 2 * HC] = np.broadcast_to(b2[None, :], (P, HC))
    return cb


def kernel(times, coeff_a, coeff_b, coeff_c, coeff_d, W1, b1, W2, b2, Wl, bl):
    W1 = np.asarray(W1, np.float32)
    b1 = np.asarray(b1, np.float32)
    W2 = np.asarray(W2, np.float32)
    b2 = np.asarray(b2, np.float32)
    Wl = np.asarray(Wl, np.float32)
    bl = np.asarray(bl, np.float32)

    nsteps, dxc = _host_prep(times, coeff_b, coeff_c, coeff_d)
    with_b2 = bool(np.any(b2))
    cblob = _make_cblob(W1, b1, W2, b2, with_b2)

    nc = _get_program(nsteps, with_b2)
    in_maps = [{"dx": dxc[cid], "cblob": cblob} for cid in range(NCORES)]

    from concourse.bass_utils import run_bass_kernel_spmd

    res = run_bass_kernel_spmd(nc, in_maps, list(range(NCORES)))
    z = np.stack([res.results[cid]["zout"] for cid in range(NCORES)])  # (8,128,256)
    zfull = (
        z.reshape(NCORES, P, NTILES, H).transpose(0, 2, 1, 3).reshape(B, H)
    )
    out = zfull.astype(np.float32) @ Wl + bl
    return out.astype(np.float32)



# revision 10
# speedup vs baseline: 1.1787x; 1.1787x over previous
"""Neural CDE (RK4 scan over spline-interpolated path) on 8 Trainium2 cores.

Strategy:
  - Pure batch data-parallelism: B=8192 -> 8 cores x 1024 elements.
  - Host precomputes the per-step, per-RK4-stage spline derivatives
    dX (b + c*f + d*f^2 at f in {0, dt/2, dt}), pre-scaled by the RK4
    step factors so the device glue is pure adds:
        k1' = (dt/2) k1   (z2 = z + k1')
        k2' = (dt/2) k2   (z3 = z + k2')
        k3' = dt     k3   (z4 = z + k3')
        k4' = (dt/6) k4
        z_next = z + (k1' + k3')/3 + (2/3) k2' + k4'
  - On device, per core: 2 gangs x 4 tiles x 128 batch.  Per stage:
      PE transpose z (batch-major -> feature-major), mm1 (W1 stationary,
      bias row folded via ones-row augmentation, K=33), tanh on ACT,
      mm2 (h as stationary, W2 moving) -> g (batch-major, 192 cols),
      einsum g . dx via DVE tensor_tensor with a stride-0 broadcast view
      + grouped tensor_reduce over the 6 input channels,
      RK4 glue adds on GPSIMD.
  - Final readout (z @ Wl + bl) on host (tiny).
"""

import numpy as np

B, L, C, H, MID = 8192, 256, 6, 32, 128
NCORES = 8
PCORE = B // NCORES        # 1024
P = 128
NTILES = PCORE // P        # 8
NGANGS = 2
TPG = NTILES // NGANGS     # 4
HC = H * C                 # 192
NSTEPS = L - 1             # 255

_PROG_CACHE = {}


def _dedup_sync_waits(nc):
    """Merge duplicate semaphore waits on each instruction.

    Tile's sem-assignment can emit two waits on the same semaphore (loop-entry
    + in-body); walrus codegen has a small per-instruction wait capacity, so
    collapse duplicates (sem >= a AND sem >= b  ==  sem >= max(a, b))."""
    import concourse.mybir as mybir

    for f in nc.m.functions:
        for bb in f.blocks:
            for inst in bb.instructions:
                si = inst.sync_info
                if si is None or not si.on_wait or len(si.on_wait) <= 1:
                    continue
                seen = {}
                order = []
                mergeable = True
                for w in si.on_wait:
                    key = (w.sync_type, w.id, w.wait_mode, w.wait_reg)
                    if key in seen and w.wait_mode == "sem-ge-imm":
                        if (w.wait_value or 0) > (seen[key].wait_value or 0):
                            seen[key] = w
                    elif key in seen:
                        mergeable = False
                        break
                    else:
                        seen[key] = w
                        order.append(key)
                if mergeable and len(order) != len(si.on_wait):
                    inst.sync_info = mybir.SyncInfo(
                        on_wait=[seen[k] for k in order], on_update=si.on_update
                    )


def _spill_sync_waits(nc):
    """Walrus codegen has a small per-instruction sync-wait capacity
    (~3 for LDW-carrying Matmult, ~4 elsewhere).  Move excess waits onto
    same-engine NoOps inserted immediately before the instruction —
    engine program order makes this equivalent."""
    import concourse.mybir as mybir

    def limit_for(inst):
        return 1

    for f in nc.m.functions:
        for bb in f.blocks:
            il = bb.instructions
            new = []
            changed = False
            for inst in il:
                si = inst.sync_info
                waits = list(si.on_wait) if (si and si.on_wait) else []
                lim = limit_for(inst)
                if len(waits) > lim:
                    excess = waits[: len(waits) - lim]
                    keep = waits[len(waits) - lim :]
                    for i in range(0, len(excess), 1):
                        nop = mybir.InstNoOp(
                            name=f"{inst.name}-wsp{i}",
                            engine=inst.engine,
                            sync_info=mybir.SyncInfo(
                                on_wait=excess[i : i + 1], on_update=[]
                            ),
                            bass_nofuse=True,
                        )
                        new.append(nop)
                    inst.sync_info = mybir.SyncInfo(
                        on_wait=keep, on_update=si.on_update
                    )
                    changed = True
                new.append(inst)
            if changed:
                bb.instructions = new


def _build_program(nsteps, with_b2, use_for_i=True, spill=True):
    from contextlib import ExitStack

    import concourse.bass as bass
    import concourse.mybir as mybir
    from concourse.tile import TileContext

    f32 = mybir.dt.float32
    bf16 = mybir.dt.bfloat16
    AF = mybir.ActivationFunctionType
    ALU = mybir.AluOpType

    nc = bass.Bass()
    CB = 2 * P + HC  # w1aug(128) + ident(128) + w2(192), all bf16
    dx_d = nc.declare_dram_parameter("dx", [nsteps, P, NTILES * 4 * C], f32, False)
    cb_d = nc.declare_dram_parameter("cblob", [P, CB], bf16, False)
    idf_d = nc.declare_dram_parameter("idf", [P, P], f32, False)
    if with_b2:
        b2_d = nc.declare_dram_parameter("b2bc", [P, HC], f32, False)
    zout_d = nc.declare_dram_parameter("zout", [P, NTILES * H], f32, True)

    with TileContext(nc) as tc, ExitStack() as ctx:
        const = ctx.enter_context(tc.tile_pool(name="const", bufs=1))
        state = ctx.enter_context(tc.tile_pool(name="state", bufs=1))
        dxpool = ctx.enter_context(tc.tile_pool(name="dxp", bufs=2))
        hpool = ctx.enter_context(tc.tile_pool(name="hsb", bufs=2))
        tpool = ctx.enter_context(tc.tile_pool(name="tsb", bufs=4))
        kpool = ctx.enter_context(tc.tile_pool(name="ksb", bufs=2))
        zspool = ctx.enter_context(tc.tile_pool(name="zs", bufs=2))
        scpool = ctx.enter_context(tc.tile_pool(name="sc", bufs=2))
        hppool = ctx.enter_context(tc.tile_pool(name="hp", bufs=2, space="PSUM"))
        gppool = ctx.enter_context(tc.tile_pool(name="gp", bufs=4, space="PSUM"))
        tppool = ctx.enter_context(tc.tile_pool(name="tp", bufs=2, space="PSUM"))

        cb_s = const.tile([P, CB], bf16, tag="cblob")
        nc.sync.dma_start(cb_s[:], cb_d[:])
        w1_s = cb_s[0 : H + 1, 0:MID]
        id_s = cb_s[:, P : 2 * P]
        w2_s = cb_s[:, 2 * P : 2 * P + HC]
        idf_s = const.tile([P, P], f32, tag="idf")
        nc.sync.dma_start(idf_s[:], idf_d[:])
        if with_b2:
            b2_s = const.tile([P, HC], f32, tag="b2bc")
            nc.sync.dma_start(b2_s[:], b2_d[:])
            b2v = (
                b2_s[:]
                .rearrange("p (h i) -> p h i", i=C)
                .unsqueeze(1)
                .broadcast_to((P, 2, H, C))
            )

        # Warm reads: consume every const on PE once before the loop so its
        # observed tick for the const DMAs advances and in-loop matmuls don't
        # burn a wait slot re-waiting on it every iteration.
        warm_ps = tppool.tile([P, P], f32, tag="tp")
        nc.tensor.matmul(warm_ps[:], id_s, w2_s[:, 0:P], start=True, stop=True)
        warm_ps2 = tppool.tile([P, P], f32, tag="tp")
        nc.tensor.matmul(warm_ps2[:], idf_s, idf_s, start=True, stop=True, is_transpose=True)
        zz = []
        for g in range(NGANGS):
            zt_ = state.tile([P, TPG * H], f32, tag=f"zz{g}")
            nc.vector.memset(zt_[:], 0.0)
            zz.append(zt_)
        zTs = []
        for g in range(NGANGS):
            pair = []
            for nm in range(2):
                t = state.tile([H + 1, TPG * P], bf16, tag=f"zT{g}_{nm}")
                nc.vector.memset(t[H : H + 1, :], 1.0)  # ones row (bias)
                pair.append(t)
            zTs.append(pair)

        def emit_step(dxt):
            for g in range(NGANGS):
                kk = kpool.tile([P, 4 * TPG * H], f32, tag=f"kk{g}")
                zstage = zz[g]
                for s in range(4):
                    # --- transpose stage-z to feature-major (33, 512) ---
                    tp_t = tppool.tile([H, TPG * P], f32, tag="tp")
                    for c in range(TPG):
                        nc.tensor.matmul(
                            tp_t[:, c * P : (c + 1) * P],
                            zstage[:, c * H : (c + 1) * H],
                            idf_s,
                            start=(c == 0),
                            stop=(c == TPG - 1),
                            is_transpose=True,
                        )
                    zT = zTs[g][s % 2]
                    nc.scalar.activation(zT[0:H, :], tp_t[:], AF.Copy)
                    # --- mm1 + tanh -> hT (feature-major, SBUF) ---
                    hp_t = hppool.tile([MID, TPG * P], f32, tag="hp")
                    nc.tensor.matmul(hp_t[:], w1_s, zT[:], start=True, stop=True)
                    hT = hpool.tile([MID, TPG * P], bf16, tag="hT")
                    nc.scalar.activation(hT[:], hp_t[:], AF.Tanh)
                    # --- mm2 (2 tiles per psum bank) + einsum ---
                    for j in range(2):
                        gp_t = gppool.tile([P, 2 * HC], f32, tag="gp")
                        for u in range(2):
                            c = 2 * j + u
                            nc.tensor.matmul(
                                gp_t[:, u * HC : (u + 1) * HC],
                                hT[:, c * P : (c + 1) * P],
                                w2_s,
                                start=(u == 0),
                                stop=(u == 1),
                            )
                        T0 = g * TPG + 2 * j
                        dxv = (
                            dxt[:]
                            .rearrange("p (t v i) -> p t v i", t=NTILES, v=4)[
                                :, T0 : T0 + 2, s, :
                            ]
                            .unsqueeze(2)
                            .broadcast_to((P, 2, H, C))
                        )
                        gv = gp_t[:].rearrange("p (u h i) -> p u h i", u=2, i=C)
                        t_t = tpool.tile([P, 2 * HC], f32, tag="tt")
                        tv = t_t[:].rearrange("p (u h i) -> p u h i", u=2, i=C)
                        if with_b2:
                            g2 = tpool.tile([P, 2 * HC], f32, tag="g2")
                            g2v = g2[:].rearrange("p (u h i) -> p u h i", u=2, i=C)
                            nc.vector.tensor_tensor(g2v, gv, b2v, ALU.add)
                            nc.vector.tensor_tensor(tv, g2v, dxv, ALU.mult)
                        else:
                            nc.vector.tensor_tensor(tv, gv, dxv, ALU.mult)
                        ksl = kk[
                            :, s * TPG * H + j * 2 * H : s * TPG * H + (j + 1) * 2 * H
                        ]
                        nc.vector.tensor_reduce(
                            ksl, tv, axis=mybir.AxisListType.X, op=ALU.add
                        )
                    if s < 3:
                        znew = zspool.tile([P, TPG * H], f32, tag=f"zst{g}")
                        nc.vector.tensor_tensor(
                            znew[:],
                            zz[g][:],
                            kk[:, s * TPG * H : (s + 1) * TPG * H],
                            ALU.add,
                        )
                        zstage = znew
                # --- RK4 combine: z += (k1+k3)/3 + (2/3) k2 + k4 ---
                k1 = kk[:, 0 * TPG * H : 1 * TPG * H]
                k2 = kk[:, 1 * TPG * H : 2 * TPG * H]
                k3 = kk[:, 2 * TPG * H : 3 * TPG * H]
                k4 = kk[:, 3 * TPG * H : 4 * TPG * H]
                a = scpool.tile([P, TPG * H], f32, tag=f"cmb{g}")
                nc.vector.tensor_tensor(a[:], k1, k3, ALU.add)
                bb = scpool.tile([P, TPG * H], f32, tag=f"cmb2{g}")
                nc.vector.scalar_tensor_tensor(
                    bb[:], a[:], 1.0 / 3.0, k4, ALU.mult, ALU.add
                )
                cc = scpool.tile([P, TPG * H], f32, tag=f"cmb3{g}")
                nc.vector.scalar_tensor_tensor(
                    cc[:], k2, 2.0 / 3.0, bb[:], ALU.mult, ALU.add
                )
                nc.vector.tensor_tensor(zz[g][:], cc[:], zz[g][:], ALU.add)

        if use_for_i:
            tc.strict_bb_all_engine_barrier()
            with tc.For_i(0, nsteps, 1, staggered_reset=True) as iv:
                dxt = dxpool.tile([P, NTILES * 4 * C], f32, tag="dx")
                nc.sync.dma_start(dxt[:], dx_d[bass.ds(iv, 1)].squeeze(0))
                emit_step(dxt)
        else:
            for it in range(nsteps):
                dxt = dxpool.tile([P, NTILES * 4 * C], f32, tag="dx")
                nc.gpsimd.dma_start(dxt[:], dx_d[it])
                emit_step(dxt)

        if use_for_i:
            tc.strict_bb_all_engine_barrier()
        for g in range(NGANGS):
            nc.sync.dma_start(zout_d[:, g * TPG * H : (g + 1) * TPG * H], zz[g][:])
    _dedup_sync_waits(nc)
    if spill:
        _spill_sync_waits(nc)
    return nc


def _get_program(nsteps, with_b2, use_for_i=True):
    key = (nsteps, with_b2, use_for_i)
    if key not in _PROG_CACHE:
        _PROG_CACHE[key] = _build_program(nsteps, with_b2, use_for_i)
    return _PROG_CACHE[key]


def _host_prep(times, coeff_b, coeff_c, coeff_d):
    """-> (nsteps, dxc) with dxc shaped (NCORES, nsteps, P, NTILES*4*C)."""
    times = np.asarray(times, np.float32)
    b_ = np.asarray(coeff_b, np.float32)
    c_ = np.asarray(coeff_c, np.float32)
    d_ = np.asarray(coeff_d, np.float32)
    dts = (times[1:] - times[:-1]).astype(np.float32)  # (nsteps,)
    nsteps = dts.shape[0]
    fm = (dts / 2).astype(np.float32)
    dx0 = b_
    dxm = b_ + c_ * fm[None, :, None] + d_ * (fm * fm)[None, :, None]
    dx1 = b_ + c_ * dts[None, :, None] + d_ * (dts * dts)[None, :, None]
    sA = (dts / 2)[None, :, None].astype(np.float32)
    sC = dts[None, :, None].astype(np.float32)
    sD = (dts / 6)[None, :, None].astype(np.float32)
    dxall = np.stack(
        [dx0 * sA, dxm * sA, dxm * sC, dx1 * sD], axis=2
    )  # (B, nsteps, 4, C)
    dxc = (
        dxall.reshape(NCORES, NTILES, P, nsteps, 4, C)
        .transpose(0, 3, 2, 1, 4, 5)
        .reshape(NCORES, nsteps, P, NTILES * 4 * C)
        .astype(np.float32)
    )
    return nsteps, np.ascontiguousarray(dxc)


def _make_cblob(W1, b1, W2):
    """(128, CB) bf16 const blob: [w1aug pad | I128 | W2]."""
    import ml_dtypes

    CB = 2 * P + HC
    cb = np.zeros((P, CB), np.float32)
    cb[0:H, 0:MID] = W1
    cb[H, 0:MID] = b1
    cb[:, P : 2 * P] = np.eye(P, dtype=np.float32)
    cb[0:MID, 2 * P : 2 * P + HC] = W2
    return cb.astype(ml_dtypes.bfloat16)


def make_in_maps(times, coeff_b, coeff_c, coeff_d, W1, b1, W2, b2):
    """-> (nsteps, with_b2, in_maps) shared by kernel() and test harnesses."""
    W1 = np.asarray(W1, np.float32)
    b1 = np.asarray(b1, np.float32)
    W2 = np.asarray(W2, np.float32)
    b2 = np.asarray(b2, np.float32)
    nsteps, dxc = _host_prep(times, coeff_b, coeff_c, coeff_d)
    with_b2 = bool(np.any(b2))
    cblob = _make_cblob(W1, b1, W2)
    idf = np.eye(P, dtype=np.float32)
    in_maps = []
    for cid in range(NCORES):
        m = {"dx": dxc[cid], "cblob": cblob, "idf": idf}
        if with_b2:
            m["b2bc"] = np.ascontiguousarray(
                np.broadcast_to(b2[None, :], (P, HC)).astype(np.float32)
            )
        in_maps.append(m)
    return nsteps, with_b2, in_maps


def kernel(times, coeff_a, coeff_b, coeff_c, coeff_d, W1, b1, W2, b2, Wl, bl):
    Wl = np.asarray(Wl, np.float32)
    bl = np.asarray(bl, np.float32)

    nsteps, with_b2, in_maps = make_in_maps(times, coeff_b, coeff_c, coeff_d, W1, b1, W2, b2)
    nc = _get_program(nsteps, with_b2)

    from concourse.bass_utils import run_bass_kernel_spmd

    res = run_bass_kernel_spmd(nc, in_maps, list(range(NCORES)))
    z = np.stack([res.results[cid]["zout"] for cid in range(NCORES)])  # (8,128,256)
    zfull = (
        z.reshape(NCORES, P, NTILES, H).transpose(0, 2, 1, 3).reshape(B, H)
    )
    out = zfull.astype(np.float32) @ Wl + bl
    return out.astype(np.float32)

